# revision 21
# baseline (speedup 1.0000x reference)
"""Self-contained Bass/Trainium2 kernel for single-head causal self-attention.

reference semantics (fp32):
  qkv = x @ Wqkv; q,k,v = split(qkv)
  att = softmax(causal(q k^T / sqrt(C)))
  y = (att @ v) @ Wproj

Sharding: 8 cores = 4 batches x 2 causally-balanced query-tile sets.
Program A (cores 0-3): q-tiles {0..7, 24..31} of its batch.
Program B (cores 4-7): q-tiles {8..23} of its batch.
Each runs as its own NEFF on a disjoint 4-device mesh, dispatched
concurrently.

Layout: host pre-transposes x to x^T (bf16), so no on-device
transposes anywhere.  S^T = K^T-chunks.T @ Q^T (keys on partitions) so
softmax needs no max pass; row sums via a ones-column matmul.  P.V is
computed as o^T = V.T @ P^T (channels on partitions) so the projection
consumes it directly with no transpose; 1/l scaling is applied to the
projected output (projection is linear).

Precision: phase 1 (QKV) and the diagonal / masked attention blocks run
in bf16 (hw speed identical to fp32r); off-diagonal attention blocks run
in fp8(e4m3) with DoubleRow perf mode (2 key/contraction tiles per
instruction at 0.5 cycles/row).  Off-diagonal fp8 errors are attenuated
by ~1/sqrt(n_keys) through the softmax average, keeping the overall
error well inside the 2e-2 budget.  Diagonal S matmuls only compute the
unmasked query range (partial-width).
"""

import sys

sys.path.insert(0, "/opt/trn_rl_repo")

import numpy as np
import ml_dtypes

B, T, C = 4, 4096, 512
N_CORES = 8
SCALE = 1.0 / np.sqrt(C)
MASKVAL = -1.0e10
EXPB = -4.0    # constant exp bias; cancels in softmax, keeps p in fp8 range

GROUPS_A = [0, 24, 28]       # group base tile (tiles a..a+3), program A
GROUPS_B = [4, 8, 12, 16, 20]
KV_CHUNKS_A = 8              # 512-row x chunks needed for K/V
KV_CHUNKS_B = 6
Q_CHUNKS_A = [0, 6, 7]       # x chunks holding the program's q rows
Q_CHUNKS_B = [1, 2, 3, 4, 5]

_CACHE = {}


def _dmask_np():
    # [128, 4*512] additive masks for the 4 diagonal-offset variants.
    # Variant d, sub-tile k columns: k<d fully masked, k==d triangular
    # (valid where j' <= i'), k>d fully visible.
    m = np.zeros((128, 4, 4, 128), dtype=np.float32)
    jj = np.arange(128)[:, None]
    ii = np.arange(128)[None, :]
    tri = np.where(jj <= ii, 0.0, MASKVAL).astype(np.float32)
    for d in range(4):
        for k in range(4):
            if k < d:
                m[:, d, k, :] = MASKVAL
            elif k == d:
                m[:, d, k, :] = tri
    return m.reshape(128, 4 * 512)


def _build(group_starts, kv_chunks, q_chunks):
    TQ = 512 * len(group_starts)
    import concourse.mybir as mybir
    import concourse.tile as tile
    from concourse import bacc

    F32 = mybir.dt.float32
    BF16 = mybir.dt.bfloat16
    F8 = mybir.dt.float8e4
    AF = mybir.ActivationFunctionType
    DR = mybir.MatmulPerfMode.DoubleRow
    TKV = kv_chunks * 512
    NT = kv_chunks * 4           # number of 128-row key tiles held

    nc = bacc.Bacc("TRN2", target_bir_lowering=False, debug=False,
                   num_devices=4)

    xT_in = nc.dram_tensor("xT_in", [C, T], BF16, kind="ExternalInput").ap()
    wqkv_in = nc.dram_tensor("wqkv", [C, 3 * C], BF16,
                             kind="ExternalInput").ap()
    wproj_in = nc.dram_tensor("wproj", [C, C], BF16,
                              kind="ExternalInput").ap()
    wp8_in = nc.dram_tensor("wp8", [C, C], F8, kind="ExternalInput").ap()
    fp8_q = group_starts[0] != 0     # program B: fp8-computed Q is in budget
    if fp8_q:
        x8_in = nc.dram_tensor("x8_in", [C, T], F8,
                               kind="ExternalInput").ap()
        w8_in = nc.dram_tensor("w8", [C, 3 * C], F8,
                               kind="ExternalInput").ap()
    y_out = nc.dram_tensor("y", [TQ, C], F32, kind="ExternalOutput").ap()

    dmask_d = nc.inline_tensor(_dmask_np(), name="dmask").ap()
    ones8_d = nc.inline_tensor(
        np.ones((128, 2, 2), dtype=ml_dtypes.float8_e4m3), name="ones8").ap()
    onesb_d = nc.inline_tensor(
        np.ones((128, 2), dtype=ml_dtypes.bfloat16), name="onesb").ap()

    # chunk that must stay high-precision: the one holding rows 0..511
    # (program A's group 0); everything else tolerates fp8 compute.
    bf16_chunk = q_chunks[0] if group_starts[0] == 0 else None

    with tile.TileContext(nc) as tc:
        with tc.tile_pool(name="persist", bufs=1) as pp:
            kTb = pp.tile([128, 4, 512], BF16)      # group-0 K^T (bf16)
            kT8 = pp.tile([128, 4, TKV], F8)        # all K^T, fp8
            qTb = pp.tile([128, 4, 512], BF16)      # group-0 Q^T (bf16)
            qT8 = pp.tile([128, 4, TQ], F8)
            vb = pp.tile([128, 4, 512], BF16)       # group-0 V (bf16)
            v8 = pp.tile([128, NT, 512], F8)        # all V, fp8
            wproj_sb = pp.tile([128, 4, C], BF16)
            wp8_sb = pp.tile([128, 4, C], F8)
            dm_sb = pp.tile([128, 4, 512], F32)     # diagonal masks
            ones8 = pp.tile([128, 2, 2], F8)
            onesb = pp.tile([128, 2], BF16)
            expb = pp.tile([128, 1], F32)
            nc.vector.memset(expb[:], EXPB)

            nc.sync.dma_start(dm_sb[:],
                              dmask_d.rearrange("p (d n) -> p d n", d=4))
            nc.sync.dma_start(ones8[:], ones8_d[:])
            nc.sync.dma_start(onesb[:], onesb_d[:])
            nc.sync.dma_start(wproj_sb[:],
                              wproj_in.rearrange("(k p) f -> p k f", p=128))
            nc.sync.dma_start(wp8_sb[:],
                              wp8_in.rearrange("(k p) f -> p k f", p=128))

            # ---------------- Phase 1: K^T, Q^T, V ----------------
            with tc.tile_pool(name="wq", bufs=1) as wq_pool:
                wqkv_sb = wq_pool.tile([128, 4, 3 * C], BF16)
                nc.sync.dma_start(
                    wqkv_sb[:], wqkv_in.rearrange("(k p) f -> p k f", p=128))
                if fp8_q:
                    w8q_sb = wq_pool.tile([128, 4, C], F8)
                    nc.sync.dma_start(
                        w8q_sb[:],
                        w8_in[:, 0:C].rearrange("(k p) f -> p k f", p=128))
                with tc.tile_pool(name="p1", bufs=4) as p1, \
                     tc.tile_pool(name="p1ps", bufs=3, space="PSUM") as p1ps:
                    for tch in range(kv_chunks):
                        xT_t = p1.tile([128, 4, 512], BF16, tag="xT")
                        nc.sync.dma_start(
                            xT_t[:],
                            xT_in[:, 512 * tch:512 * (tch + 1)]
                            .rearrange("(k p) t -> p k t", p=128))
                        is_q = tch in q_chunks
                        slot = q_chunks.index(tch) if is_q else -1
                        g0 = (tch == bf16_chunk)
                        for f in range(4):
                            ps_k = p1ps.tile([128, 512], F32, tag="kf")
                            for c in range(4):
                                nc.tensor.matmul(
                                    ps_k[:],
                                    wqkv_sb[:, c, C + 128 * f:C + 128 * (f + 1)],
                                    xT_t[:, c, :],
                                    start=(c == 0), stop=(c == 3))
                            nc.vector.tensor_copy(
                                kT8[:, f, 512 * tch:512 * (tch + 1)], ps_k[:])
                            if g0:
                                nc.scalar.copy(kTb[:, f, :], ps_k[:])
                        if is_q and fp8_q:
                            x8_t = p1.tile([128, 4, 512], F8, tag="x8",
                                           bufs=2)
                            nc.sync.dma_start(
                                x8_t[:],
                                x8_in[:, 512 * tch:512 * (tch + 1)]
                                .rearrange("(k p) t -> p k t", p=128))
                            for f in range(4):
                                ps_q = p1ps.tile([128, 512], F32, tag="kf")
                                for c2 in range(2):
                                    nc.tensor.matmul(
                                        ps_q[:],
                                        w8q_sb[:, 2 * c2:2 * c2 + 2,
                                               128 * f:128 * (f + 1)],
                                        x8_t[:, 2 * c2:2 * c2 + 2, :],
                                        start=(c2 == 0), stop=(c2 == 1),
                                        perf_mode=DR)
                                nc.vector.tensor_copy(
                                    qT8[:, f, 512 * slot:512 * (slot + 1)],
                                    ps_q[:])
                        elif is_q:
                            for f in range(4):
                                ps_q = p1ps.tile([128, 512], F32, tag="kf")
                                for c in range(4):
                                    nc.tensor.matmul(
                                        ps_q[:],
                                        wqkv_sb[:, c, 128 * f:128 * (f + 1)],
                                        xT_t[:, c, :],
                                        start=(c == 0), stop=(c == 3))
                                if g0:
                                    nc.vector.tensor_copy(qTb[:, f, :],
                                                          ps_q[:])
                                    nc.gpsimd.tensor_copy(
                                        qT8[:, f,
                                            512 * slot:512 * (slot + 1)],
                                        qTb[:, f, :])
                                else:
                                    nc.vector.tensor_copy(
                                        qT8[:, f,
                                            512 * slot:512 * (slot + 1)],
                                        ps_q[:])
                        for n in range(4):
                            ps_v = p1ps.tile([128, 512], F32, tag="v")
                            for c in range(4):
                                nc.tensor.matmul(
                                    ps_v[:],
                                    xT_t[:, c, 128 * n:128 * (n + 1)],
                                    wqkv_sb[:, c, 2 * C:3 * C],
                                    start=(c == 0), stop=(c == 3))
                            nc.scalar.copy(v8[:, 4 * tch + n, :], ps_v[:])
                            if g0:
                                nc.vector.tensor_copy(vb[:, n, :], ps_v[:])

            # ---------------- Phase 2: attention + projection ----------------
            with tc.tile_pool(name="p2", bufs=1) as p2, \
                 tc.tile_pool(name="psS", bufs=3, space="PSUM") as psS, \
                 tc.tile_pool(name="psO", bufs=1, space="PSUM") as psO, \
                 tc.tile_pool(name="psl", bufs=1, space="PSUM") as psl:
                for g, a in enumerate(group_starts):
                    o_ps = [psO.tile([128, 512], F32, tag=f"o{k}",
                                     name=f"o_ps{k}") for k in range(4)]
                    l_ps = psl.tile([128, 8], F32, tag="l")
                    # ---- off-diagonal pairs, fp8 DoubleRow ----
                    # software-pipelined: emit S of pair i+1 before PV of
                    # pair i so the tensor queue covers the exp latency
                    def _emit_pv(p8_, tp_):
                        first_ = (tp_ == 0)
                        for cc in range(4):
                            nc.tensor.matmul(
                                o_ps[cc][:],
                                v8[:, tp_:tp_ + 2, 128 * cc:128 * (cc + 1)],
                                p8_[:],
                                start=first_, stop=False, perf_mode=DR)
                        for k in range(4):
                            nc.tensor.matmul(
                                l_ps[:, 2 * k:2 * (k + 1)],
                                p8_[:, :, 128 * k:128 * (k + 1)],
                                ones8[:],
                                start=(first_ and k == 0), stop=False,
                                perf_mode=DR, skip_group_check=True)

                    pend = None
                    for tp in range(0, a, 2):
                        s_pair = []
                        for h in range(2):
                            t = tp + h
                            s_ps = psS.tile([128, 512], F32, tag="s")
                            for c2 in range(2):
                                nc.tensor.matmul(
                                    s_ps[:],
                                    kT8[:, 2 * c2:2 * c2 + 2,
                                        128 * t:128 * (t + 1)],
                                    qT8[:, 2 * c2:2 * c2 + 2,
                                        512 * g:512 * (g + 1)],
                                    start=(c2 == 0), stop=(c2 == 1),
                                    perf_mode=DR)
                            s_pair.append(s_ps)
                        p8 = p2.tile([128, 2, 512], F8, tag="p8", bufs=4)
                        for h in range(2):
                            nc.scalar.activation(p8[:, h, :], s_pair[h][:],
                                                 AF.Exp, bias=expb[:],
                                                 scale=SCALE)
                        if pend is not None:
                            _emit_pv(*pend)
                        pend = (p8, tp)
                    if pend is not None:
                        _emit_pv(*pend)
                    # ---- diagonal blocks ----
                    if a == 0:
                        # bf16, partial query width (rows 0..511 need
                        # high precision)
                        for d in range(4):
                            w = 512 - 128 * d
                            qlo = 512 * g + 128 * d
                            qhi = 512 * (g + 1)
                            s_ps = psS.tile([128, 512], F32, tag="s")
                            for c in range(4):
                                nc.tensor.matmul(
                                    s_ps[:, :w],
                                    kTb[:, c, 128 * d:128 * (d + 1)],
                                    qTb[:, c, 128 * d:512],
                                    start=(c == 0), stop=(c == 3))
                            nc.vector.tensor_add(s_ps[:, :w], s_ps[:, :w],
                                                 dm_sb[:, d, 128 * d:512])
                            pd = p2.tile([128, 512], BF16, tag="pd", bufs=3)
                            nc.scalar.activation(pd[:, :w], s_ps[:, :w],
                                                 AF.Exp, bias=expb[:],
                                                 scale=SCALE)
                            start_o = (d == 0)
                            for cc in range(4):
                                nc.tensor.matmul(
                                    o_ps[cc][:, 128 * d:512],
                                    vb[:, d, 128 * cc:128 * (cc + 1)],
                                    pd[:, :w],
                                    start=start_o, stop=(d == 3),
                                    skip_group_check=True)
                            for k in range(d, 4):
                                nc.tensor.matmul(
                                    l_ps[:, 2 * k:2 * (k + 1)],
                                    pd[:, 128 * (k - d):128 * (k - d + 1)],
                                    onesb[:],
                                    start=(start_o and k == 0), stop=(k == d),
                                    skip_group_check=True)
                    else:
                        # fp8 DoubleRow pairs, full width, masked via psum add
                        for dp in range(2):
                            s_pair = []
                            for h in range(2):
                                d = 2 * dp + h
                                t = a + d
                                s_ps = psS.tile([128, 512], F32, tag="s")
                                for c2 in range(2):
                                    nc.tensor.matmul(
                                        s_ps[:],
                                        kT8[:, 2 * c2:2 * c2 + 2,
                                            128 * t:128 * (t + 1)],
                                        qT8[:, 2 * c2:2 * c2 + 2,
                                            512 * g:512 * (g + 1)],
                                        start=(c2 == 0), stop=(c2 == 1),
                                        perf_mode=DR)
                                nc.vector.tensor_add(s_ps[:], s_ps[:],
                                                     dm_sb[:, d, :])
                                s_pair.append(s_ps)
                            p8 = p2.tile([128, 2, 512], F8, tag="p8", bufs=4)
                            for h in range(2):
                                nc.scalar.activation(p8[:, h, :],
                                                     s_pair[h][:], AF.Exp,
                                                     bias=expb[:], scale=SCALE)
                            last = (dp == 1)
                            for cc in range(4):
                                nc.tensor.matmul(
                                    o_ps[cc][:],
                                    v8[:, a + 2 * dp:a + 2 * dp + 2,
                                       128 * cc:128 * (cc + 1)],
                                    p8[:],
                                    start=False, stop=last, perf_mode=DR)
                            for k in range(4):
                                nc.tensor.matmul(
                                    l_ps[:, 2 * k:2 * (k + 1)],
                                    p8[:, :, 128 * k:128 * (k + 1)],
                                    ones8[:],
                                    start=False, stop=last,
                                    perf_mode=DR, skip_group_check=True)
                    # ---- epilogue: o^T -> projection -> 1/l scale ----
                    if a == 0:
                        oT = p2.tile([128, 4, 512], BF16, tag="oT", bufs=2)
                    else:
                        oT = p2.tile([128, 4, 512], F8, tag="oT8", bufs=2)
                    for cc in range(4):
                        if cc % 2 == 0:
                            nc.scalar.copy(oT[:, cc, :], o_ps[cc][:])
                        else:
                            nc.vector.tensor_copy(oT[:, cc, :], o_ps[cc][:])
                    for k in range(4):
                        r_sb = p2.tile([128, 1], F32, tag="r", bufs=2)
                        nc.vector.reciprocal(r_sb[:], l_ps[:, 2 * k:2 * k + 1])
                        y_ps = psS.tile([128, 512], F32, tag="s")
                        if a == 0:
                            for u in range(4):
                                nc.tensor.matmul(
                                    y_ps[:],
                                    oT[:, u, 128 * k:128 * (k + 1)],
                                    wproj_sb[:, u, :],
                                    start=(u == 0), stop=(u == 3))
                        else:
                            for u2 in range(2):
                                nc.tensor.matmul(
                                    y_ps[:],
                                    oT[:, 2 * u2:2 * u2 + 2,
                                       128 * k:128 * (k + 1)],
                                    wp8_sb[:, 2 * u2:2 * u2 + 2, :],
                                    start=(u2 == 0), stop=(u2 == 1),
                                    perf_mode=DR)
                        y_sb = p2.tile([128, 512], F32, tag="ysb", bufs=2)
                        nc.vector.tensor_scalar_mul(y_sb[:], y_ps[:], r_sb[:])
                        r0 = 128 * (4 * g + k)
                        nc.sync.dma_start(y_out[r0:r0 + 128, :], y_sb[:])
    nc.compile()
    return nc


def _make_runner(nc, devices):
    """Jitted shard_map runner for one program over a 4-device mesh.

    Mirrors bass2jax.run_bass_via_pjrt's multi-core branch, but with an
    explicit device list so two programs can run concurrently on
    disjoint meshes.
    """
    import jax
    import concourse.mybir as mybir
    from concourse.bass2jax import _bass_exec_p, install_neuronx_cc_hook
    from jax.experimental.shard_map import shard_map
    from jax.sharding import Mesh, PartitionSpec

    from concourse.bass2jax import partition_id_tensor

    install_neuronx_cc_hook()

    partition_name = (nc.partition_id_tensor.name
                      if nc.partition_id_tensor else None)
    in_names, out_names, out_avals, zero_outs = [], [], [], []
    for alloc in nc.m.functions[0].allocations:
        if not isinstance(alloc, mybir.MemoryLocationSet):
            continue
        name = alloc.memorylocations[0].name
        if alloc.kind == "ExternalInput":
            if name != partition_name:
                in_names.append(name)
        elif alloc.kind == "ExternalOutput":
            out_names.append(name)
            shape = tuple(alloc.tensor_shape)
            dtype = mybir.dt.np(alloc.dtype)
            out_avals.append(jax.core.ShapedArray(shape, dtype))
            zero_outs.append(np.zeros(shape, dtype))
    n_params = len(in_names)
    n_outs = len(out_avals)
    all_names = in_names + out_names
    if partition_name is not None:
        all_names = all_names + [partition_name]
    donate = tuple(range(n_params, n_params + n_outs))
    n_cores = len(devices)

    def _body(*args):
        operands = list(args)
        if partition_name is not None:
            operands.append(partition_id_tensor())
        outs = _bass_exec_p.bind(
            *operands,
            out_avals=tuple(out_avals),
            in_names=tuple(all_names),
            out_names=tuple(out_names),
            lowering_input_output_aliases=(),
            sim_require_finite=True,
            sim_require_nnan=True,
            nc=nc,
        )
        return tuple(outs)

    mesh = Mesh(np.asarray(devices), ("core",))
    in_specs = (PartitionSpec("core"),) * (n_params + n_outs)
    out_specs = (PartitionSpec("core"),) * n_outs
    sharded = jax.jit(
        shard_map(_body, mesh=mesh, in_specs=in_specs, out_specs=out_specs,
                  check_rep=False),
        donate_argnums=donate, keep_unused=True)

    def run(in_maps):
        per_core = [[np.asarray(m[name]) for name in in_names] for m in in_maps]
        concat_in = [
            np.concatenate([per_core[c][i] for c in range(n_cores)], axis=0)
            for i in range(n_params)
        ]
        concat_zeros = [
            np.zeros((n_cores * z.shape[0], *z.shape[1:]), z.dtype)
            for z in zero_outs
        ]
        return sharded(*concat_in, *concat_zeros)  # async jax arrays

    def gather(out_arrs):
        return [
            {name: np.asarray(out_arrs[i]).reshape(n_cores, *out_avals[i].shape)[c]
             for i, name in enumerate(out_names)}
            for c in range(n_cores)
        ]

    return run, gather, out_names


def _tiles_for(group_starts):
    tiles = []
    for a in group_starts:
        tiles.extend(range(a, a + 4))
    return tiles


def _get_runners():
    if "runA" not in _CACHE:
        import jax
        devs = jax.devices()
        ncA = _build(GROUPS_A, KV_CHUNKS_A, Q_CHUNKS_A)
        ncB = _build(GROUPS_B, KV_CHUNKS_B, Q_CHUNKS_B)
        _CACHE["runA"] = _make_runner(ncA, devs[0:4])
        _CACHE["runB"] = _make_runner(ncB, devs[4:8])
    return _CACHE["runA"], _CACHE["runB"]


def kernel(x, Wqkv, Wproj, _trace_ctx=None):
    x = np.ascontiguousarray(x, dtype=np.float32)
    xT = np.ascontiguousarray(np.transpose(x, (0, 2, 1)))  # [B, C, T]
    xT_bf = xT.astype(ml_dtypes.bfloat16)
    xT_f8 = xT.astype(ml_dtypes.float8_e4m3)
    wqkv_f8 = np.asarray(Wqkv, dtype=np.float32).astype(ml_dtypes.float8_e4m3)
    wqkv_bf = np.asarray(Wqkv, dtype=np.float32).astype(ml_dtypes.bfloat16)
    wproj_bf = np.asarray(Wproj, dtype=np.float32).astype(ml_dtypes.bfloat16)
    wproj_f8 = np.asarray(Wproj, dtype=np.float32).astype(ml_dtypes.float8_e4m3)

    (runA, gatherA, _), (runB, gatherB, _) = _get_runners()

    maps = [{"xT_in": xT_bf[b], "x8_in": xT_f8[b], "wqkv": wqkv_bf,
             "w8": wqkv_f8, "wproj": wproj_bf, "wp8": wproj_f8}
            for b in range(B)]

    import contextlib
    ctx = _trace_ctx if _trace_ctx is not None else contextlib.nullcontext()
    with ctx:
        outA = runA(maps)
        outB = runB(maps)
        resA = gatherA(outA)
        resB = gatherB(outB)

    tilesA = _tiles_for(GROUPS_A)
    tilesB = _tiles_for(GROUPS_B)
    out = np.empty((B, T, C), dtype=np.float32)
    for b in range(B):
        for slot, tile_i in enumerate(tilesA):
            out[b, 128 * tile_i:128 * (tile_i + 1)] = \
                resA[b]["y"][128 * slot:128 * (slot + 1)]
        for slot, tile_i in enumerate(tilesB):
            out[b, 128 * tile_i:128 * (tile_i + 1)] = \
                resB[b]["y"][128 * slot:128 * (slot + 1)]
    return out


# revision 22
# speedup vs baseline: 1.0247x; 1.0247x over previous
"""Self-contained Bass/Trainium2 kernel for single-head causal self-attention.

reference semantics (fp32):
  qkv = x @ Wqkv; q,k,v = split(qkv)
  att = softmax(causal(q k^T / sqrt(C)))
  y = (att @ v) @ Wproj

Sharding: 8 cores = 4 batches x 2 causally-balanced query-tile sets.
Program A (cores 0-3): q-tiles {0..7, 24..31} of its batch.
Program B (cores 4-7): q-tiles {8..23} of its batch.
Each runs as its own NEFF on a disjoint 4-device mesh, dispatched
concurrently.

Layout: host pre-transposes x to x^T (bf16), so no on-device
transposes anywhere.  S^T = K^T-chunks.T @ Q^T (keys on partitions) so
softmax needs no max pass; row sums via a ones-column matmul.  P.V is
computed as o^T = V.T @ P^T (channels on partitions) so the projection
consumes it directly with no transpose; 1/l scaling is applied to the
projected output (projection is linear).

Precision: phase 1 (QKV) and the diagonal / masked attention blocks run
in bf16 (hw speed identical to fp32r); off-diagonal attention blocks run
in fp8(e4m3) with DoubleRow perf mode (2 key/contraction tiles per
instruction at 0.5 cycles/row).  Off-diagonal fp8 errors are attenuated
by ~1/sqrt(n_keys) through the softmax average, keeping the overall
error well inside the 2e-2 budget.  Diagonal S matmuls only compute the
unmasked query range (partial-width).
"""

import sys

sys.path.insert(0, "/opt/trn_rl_repo")

import numpy as np
import ml_dtypes

B, T, C = 4, 4096, 512
N_CORES = 8
SCALE = 1.0 / np.sqrt(C)
MASKVAL = -1.0e10
EXPB = -4.0    # constant exp bias; cancels in softmax, keeps p in fp8 range

GROUPS_A = [0, 24, 28]       # group base tile (tiles a..a+3), program A
GROUPS_B = [4, 8, 12, 16, 20]
KV_CHUNKS_A = 8              # 512-row x chunks needed for K/V
KV_CHUNKS_B = 6
Q_CHUNKS_A = [0, 6, 7]       # x chunks holding the program's q rows
Q_CHUNKS_B = [1, 2, 3, 4, 5]

_CACHE = {}


def _dmask_np():
    # [128, 4*512] additive masks for the 4 diagonal-offset variants.
    # Variant d, sub-tile k columns: k<d fully masked, k==d triangular
    # (valid where j' <= i'), k>d fully visible.
    m = np.zeros((128, 4, 4, 128), dtype=np.float32)
    jj = np.arange(128)[:, None]
    ii = np.arange(128)[None, :]
    tri = np.where(jj <= ii, 0.0, MASKVAL).astype(np.float32)
    for d in range(4):
        for k in range(4):
            if k < d:
                m[:, d, k, :] = MASKVAL
            elif k == d:
                m[:, d, k, :] = tri
    return m.reshape(128, 4 * 512)


def _build(group_starts, kv_chunks, q_chunks):
    TQ = 512 * len(group_starts)
    import concourse.mybir as mybir
    import concourse.tile as tile
    from concourse import bacc

    F32 = mybir.dt.float32
    BF16 = mybir.dt.bfloat16
    F8 = mybir.dt.float8e4
    AF = mybir.ActivationFunctionType
    DR = mybir.MatmulPerfMode.DoubleRow
    TKV = kv_chunks * 512
    NT = kv_chunks * 4           # number of 128-row key tiles held

    nc = bacc.Bacc("TRN2", target_bir_lowering=False, debug=False,
                   num_devices=4)

    xT_in = nc.dram_tensor("xT_in", [C, T], BF16, kind="ExternalInput").ap()
    wqkv_in = nc.dram_tensor("wqkv", [C, 3 * C], BF16,
                             kind="ExternalInput").ap()
    wproj_in = nc.dram_tensor("wproj", [C, C], BF16,
                              kind="ExternalInput").ap()
    wp8_in = nc.dram_tensor("wp8", [C, C], F8, kind="ExternalInput").ap()
    fp8_q = group_starts[0] != 0     # program B: fp8-computed Q is in budget
    if fp8_q:
        x8_in = nc.dram_tensor("x8_in", [C, T], F8,
                               kind="ExternalInput").ap()
        w8_in = nc.dram_tensor("w8", [C, 3 * C], F8,
                               kind="ExternalInput").ap()
    y_out = nc.dram_tensor("y", [TQ, C], F32, kind="ExternalOutput").ap()

    dmask_d = nc.inline_tensor(_dmask_np(), name="dmask").ap()
    ones8_d = nc.inline_tensor(
        np.ones((128, 2, 2), dtype=ml_dtypes.float8_e4m3), name="ones8").ap()
    onesb_d = nc.inline_tensor(
        np.ones((128, 2), dtype=ml_dtypes.bfloat16), name="onesb").ap()

    # chunk that must stay high-precision: the one holding rows 0..511
    # (program A's group 0); everything else tolerates fp8 compute.
    bf16_chunk = q_chunks[0] if group_starts[0] == 0 else None

    with tile.TileContext(nc) as tc:
        with tc.tile_pool(name="persist", bufs=1) as pp:
            kTb = pp.tile([128, 4, 512], BF16)      # group-0 K^T (bf16)
            kT8 = pp.tile([128, 4, TKV], F8)        # all K^T, fp8
            qTb = pp.tile([128, 4, 512], BF16)      # group-0 Q^T (bf16)
            qT8 = pp.tile([128, 4, TQ], F8)
            vb = pp.tile([128, 4, 512], BF16)       # group-0 V (bf16)
            v8 = pp.tile([128, NT, 512], F8)        # all V, fp8
            wproj_sb = pp.tile([128, 4, C], BF16)
            wp8_sb = pp.tile([128, 4, C], F8)
            dm_sb = pp.tile([128, 4, 512], F32)     # diagonal masks
            ones8 = pp.tile([128, 2, 2], F8)
            onesb = pp.tile([128, 2], BF16)
            expb = pp.tile([128, 1], F32)
            nc.vector.memset(expb[:], EXPB)

            nc.sync.dma_start(dm_sb[:],
                              dmask_d.rearrange("p (d n) -> p d n", d=4))
            nc.sync.dma_start(ones8[:], ones8_d[:])
            nc.sync.dma_start(onesb[:], onesb_d[:])
            nc.sync.dma_start(wproj_sb[:],
                              wproj_in.rearrange("(k p) f -> p k f", p=128))
            nc.sync.dma_start(wp8_sb[:],
                              wp8_in.rearrange("(k p) f -> p k f", p=128))

            # ---------------- Phase 1: K^T, Q^T, V ----------------
            with tc.tile_pool(name="wq", bufs=1) as wq_pool:
                wqkv_sb = wq_pool.tile([128, 4, 3 * C], BF16)
                nc.sync.dma_start(
                    wqkv_sb[:], wqkv_in.rearrange("(k p) f -> p k f", p=128))
                if fp8_q:
                    w8q_sb = wq_pool.tile([128, 4, C], F8)
                    nc.sync.dma_start(
                        w8q_sb[:],
                        w8_in[:, 0:C].rearrange("(k p) f -> p k f", p=128))
                with tc.tile_pool(name="p1", bufs=4) as p1, \
                     tc.tile_pool(name="p1ps", bufs=3, space="PSUM") as p1ps:
                    for tch in range(kv_chunks):
                        xT_t = p1.tile([128, 4, 512], BF16, tag="xT")
                        nc.sync.dma_start(
                            xT_t[:],
                            xT_in[:, 512 * tch:512 * (tch + 1)]
                            .rearrange("(k p) t -> p k t", p=128))
                        is_q = tch in q_chunks
                        slot = q_chunks.index(tch) if is_q else -1
                        g0 = (tch == bf16_chunk)
                        for f in range(4):
                            ps_k = p1ps.tile([128, 512], F32, tag="kf")
                            for c in range(4):
                                nc.tensor.matmul(
                                    ps_k[:],
                                    wqkv_sb[:, c, C + 128 * f:C + 128 * (f + 1)],
                                    xT_t[:, c, :],
                                    start=(c == 0), stop=(c == 3))
                            nc.vector.tensor_copy(
                                kT8[:, f, 512 * tch:512 * (tch + 1)], ps_k[:])
                            if g0:
                                nc.scalar.copy(kTb[:, f, :], ps_k[:])
                        if is_q and fp8_q:
                            x8_t = p1.tile([128, 4, 512], F8, tag="x8",
                                           bufs=2)
                            nc.sync.dma_start(
                                x8_t[:],
                                x8_in[:, 512 * tch:512 * (tch + 1)]
                                .rearrange("(k p) t -> p k t", p=128))
                            for f in range(4):
                                ps_q = p1ps.tile([128, 512], F32, tag="kf")
                                for c2 in range(2):
                                    nc.tensor.matmul(
                                        ps_q[:],
                                        w8q_sb[:, 2 * c2:2 * c2 + 2,
                                               128 * f:128 * (f + 1)],
                                        x8_t[:, 2 * c2:2 * c2 + 2, :],
                                        start=(c2 == 0), stop=(c2 == 1),
                                        perf_mode=DR)
                                nc.vector.tensor_copy(
                                    qT8[:, f, 512 * slot:512 * (slot + 1)],
                                    ps_q[:])
                        elif is_q:
                            for f in range(4):
                                ps_q = p1ps.tile([128, 512], F32, tag="kf")
                                for c in range(4):
                                    nc.tensor.matmul(
                                        ps_q[:],
                                        wqkv_sb[:, c, 128 * f:128 * (f + 1)],
                                        xT_t[:, c, :],
                                        start=(c == 0), stop=(c == 3))
                                if g0:
                                    nc.vector.tensor_copy(qTb[:, f, :],
                                                          ps_q[:])
                                    nc.gpsimd.tensor_copy(
                                        qT8[:, f,
                                            512 * slot:512 * (slot + 1)],
                                        qTb[:, f, :])
                                else:
                                    nc.vector.tensor_copy(
                                        qT8[:, f,
                                            512 * slot:512 * (slot + 1)],
                                        ps_q[:])
                        for n in range(4):
                            ps_v = p1ps.tile([128, 512], F32, tag="v")
                            for c in range(4):
                                nc.tensor.matmul(
                                    ps_v[:],
                                    xT_t[:, c, 128 * n:128 * (n + 1)],
                                    wqkv_sb[:, c, 2 * C:3 * C],
                                    start=(c == 0), stop=(c == 3))
                            nc.scalar.copy(v8[:, 4 * tch + n, :], ps_v[:])
                            if g0:
                                nc.vector.tensor_copy(vb[:, n, :], ps_v[:])

            # ---------------- Phase 2: attention + projection ----------------
            with tc.tile_pool(name="p2", bufs=1) as p2, \
                 tc.tile_pool(name="psS", bufs=3, space="PSUM") as psS, \
                 tc.tile_pool(name="psO", bufs=1, space="PSUM") as psO, \
                 tc.tile_pool(name="psl", bufs=1, space="PSUM") as psl:
                for g, a in enumerate(group_starts):
                    o_ps = [psO.tile([128, 512], F32, tag=f"o{k}",
                                     name=f"o_ps{k}") for k in range(4)]
                    l_ps = psl.tile([128, 8], F32, tag="l")
                    # ---- off-diagonal pairs, fp8 DoubleRow ----
                    # software-pipelined: emit S of pair i+1 before PV of
                    # pair i so the tensor queue covers the exp latency
                    def _emit_pv(p8_, tp_):
                        first_ = (tp_ == 0)
                        for cc in range(4):
                            nc.tensor.matmul(
                                o_ps[cc][:],
                                v8[:, tp_:tp_ + 2, 128 * cc:128 * (cc + 1)],
                                p8_[:],
                                start=first_, stop=False, perf_mode=DR)
                        for k in range(4):
                            nc.tensor.matmul(
                                l_ps[:, 2 * k:2 * (k + 1)],
                                p8_[:, :, 128 * k:128 * (k + 1)],
                                ones8[:],
                                start=(first_ and k == 0), stop=False,
                                perf_mode=DR, skip_group_check=True)

                    pend = None
                    for tp in range(0, a, 2):
                        s_pair = []
                        for h in range(2):
                            t = tp + h
                            s_ps = psS.tile([128, 512], F32, tag="s")
                            for c2 in range(2):
                                nc.tensor.matmul(
                                    s_ps[:],
                                    kT8[:, 2 * c2:2 * c2 + 2,
                                        128 * t:128 * (t + 1)],
                                    qT8[:, 2 * c2:2 * c2 + 2,
                                        512 * g:512 * (g + 1)],
                                    start=(c2 == 0), stop=(c2 == 1),
                                    perf_mode=DR)
                            s_pair.append(s_ps)
                        p8 = p2.tile([128, 2, 512], F8, tag="p8", bufs=4)
                        for h in range(2):
                            nc.scalar.activation(p8[:, h, :], s_pair[h][:],
                                                 AF.Exp, bias=expb[:],
                                                 scale=SCALE)
                        if pend is not None:
                            _emit_pv(*pend)
                        pend = (p8, tp)
                    # ---- diagonal blocks ----
                    if a == 0:
                        # bf16, partial query width (rows 0..511 need
                        # high precision)
                        for d in range(4):
                            w = 512 - 128 * d
                            qlo = 512 * g + 128 * d
                            qhi = 512 * (g + 1)
                            s_ps = psS.tile([128, 512], F32, tag="s")
                            for c in range(4):
                                nc.tensor.matmul(
                                    s_ps[:, :w],
                                    kTb[:, c, 128 * d:128 * (d + 1)],
                                    qTb[:, c, 128 * d:512],
                                    start=(c == 0), stop=(c == 3))
                            nc.vector.tensor_add(s_ps[:, :w], s_ps[:, :w],
                                                 dm_sb[:, d, 128 * d:512])
                            pd = p2.tile([128, 512], BF16, tag="pd", bufs=3)
                            nc.scalar.activation(pd[:, :w], s_ps[:, :w],
                                                 AF.Exp, bias=expb[:],
                                                 scale=SCALE)
                            start_o = (d == 0)
                            for cc in range(4):
                                nc.tensor.matmul(
                                    o_ps[cc][:, 128 * d:512],
                                    vb[:, d, 128 * cc:128 * (cc + 1)],
                                    pd[:, :w],
                                    start=start_o, stop=(d == 3),
                                    skip_group_check=True)
                            for k in range(d, 4):
                                nc.tensor.matmul(
                                    l_ps[:, 2 * k:2 * (k + 1)],
                                    pd[:, 128 * (k - d):128 * (k - d + 1)],
                                    onesb[:],
                                    start=(start_o and k == 0), stop=(k == d),
                                    skip_group_check=True)
                    else:
                        # fp8 DoubleRow pairs, full width, masked via psum add
                        for dp in range(2):
                            s_pair = []
                            for h in range(2):
                                d = 2 * dp + h
                                t = a + d
                                s_ps = psS.tile([128, 512], F32, tag="s")
                                for c2 in range(2):
                                    nc.tensor.matmul(
                                        s_ps[:],
                                        kT8[:, 2 * c2:2 * c2 + 2,
                                            128 * t:128 * (t + 1)],
                                        qT8[:, 2 * c2:2 * c2 + 2,
                                            512 * g:512 * (g + 1)],
                                        start=(c2 == 0), stop=(c2 == 1),
                                        perf_mode=DR)
                                nc.vector.tensor_add(s_ps[:], s_ps[:],
                                                     dm_sb[:, d, :])
                                s_pair.append(s_ps)
                            p8 = p2.tile([128, 2, 512], F8, tag="p8", bufs=4)
                            for h in range(2):
                                nc.scalar.activation(p8[:, h, :],
                                                     s_pair[h][:], AF.Exp,
                                                     bias=expb[:], scale=SCALE)
                            if dp == 0:
                                if pend is not None:
                                    _emit_pv(*pend)
                                dpend = p8
                            else:
                                for cc in range(4):
                                    nc.tensor.matmul(
                                        o_ps[cc][:],
                                        v8[:, a:a + 2,
                                           128 * cc:128 * (cc + 1)],
                                        dpend[:],
                                        start=False, stop=False,
                                        perf_mode=DR)
                                for k in range(4):
                                    nc.tensor.matmul(
                                        l_ps[:, 2 * k:2 * (k + 1)],
                                        dpend[:, :, 128 * k:128 * (k + 1)],
                                        ones8[:],
                                        start=False, stop=False,
                                        perf_mode=DR, skip_group_check=True)
                        for cc in range(4):
                            nc.tensor.matmul(
                                o_ps[cc][:],
                                v8[:, a + 2:a + 4, 128 * cc:128 * (cc + 1)],
                                p8[:],
                                start=False, stop=True, perf_mode=DR)
                        for k in range(4):
                            nc.tensor.matmul(
                                l_ps[:, 2 * k:2 * (k + 1)],
                                p8[:, :, 128 * k:128 * (k + 1)],
                                ones8[:],
                                start=False, stop=True,
                                perf_mode=DR, skip_group_check=True)
                    # ---- epilogue: o^T -> projection -> 1/l scale ----
                    if a == 0:
                        oT = p2.tile([128, 4, 512], BF16, tag="oT", bufs=2)
                    else:
                        oT = p2.tile([128, 4, 512], F8, tag="oT8", bufs=2)
                    for cc in range(4):
                        if cc % 2 == 0:
                            nc.scalar.copy(oT[:, cc, :], o_ps[cc][:])
                        else:
                            nc.vector.tensor_copy(oT[:, cc, :], o_ps[cc][:])
                    for k in range(4):
                        r_sb = p2.tile([128, 1], F32, tag="r", bufs=2)
                        nc.vector.reciprocal(r_sb[:], l_ps[:, 2 * k:2 * k + 1])
                        y_ps = psS.tile([128, 512], F32, tag="s")
                        if a == 0:
                            for u in range(4):
                                nc.tensor.matmul(
                                    y_ps[:],
                                    oT[:, u, 128 * k:128 * (k + 1)],
                                    wproj_sb[:, u, :],
                                    start=(u == 0), stop=(u == 3))
                        else:
                            for u2 in range(2):
                                nc.tensor.matmul(
                                    y_ps[:],
                                    oT[:, 2 * u2:2 * u2 + 2,
                                       128 * k:128 * (k + 1)],
                                    wp8_sb[:, 2 * u2:2 * u2 + 2, :],
                                    start=(u2 == 0), stop=(u2 == 1),
                                    perf_mode=DR)
                        y_sb = p2.tile([128, 512], F32, tag="ysb", bufs=2)
                        nc.vector.tensor_scalar_mul(y_sb[:], y_ps[:], r_sb[:])
                        r0 = 128 * (4 * g + k)
                        nc.sync.dma_start(y_out[r0:r0 + 128, :], y_sb[:])
    nc.compile()
    return nc


def _make_runner(nc, devices):
    """Jitted shard_map runner for one program over a 4-device mesh.

    Mirrors bass2jax.run_bass_via_pjrt's multi-core branch, but with an
    explicit device list so two programs can run concurrently on
    disjoint meshes.
    """
    import jax
    import concourse.mybir as mybir
    from concourse.bass2jax import _bass_exec_p, install_neuronx_cc_hook
    from jax.experimental.shard_map import shard_map
    from jax.sharding import Mesh, PartitionSpec

    from concourse.bass2jax import partition_id_tensor

    install_neuronx_cc_hook()

    partition_name = (nc.partition_id_tensor.name
                      if nc.partition_id_tensor else None)
    in_names, out_names, out_avals, zero_outs = [], [], [], []
    for alloc in nc.m.functions[0].allocations:
        if not isinstance(alloc, mybir.MemoryLocationSet):
            continue
        name = alloc.memorylocations[0].name
        if alloc.kind == "ExternalInput":
            if name != partition_name:
                in_names.append(name)
        elif alloc.kind == "ExternalOutput":
            out_names.append(name)
            shape = tuple(alloc.tensor_shape)
            dtype = mybir.dt.np(alloc.dtype)
            out_avals.append(jax.core.ShapedArray(shape, dtype))
            zero_outs.append(np.zeros(shape, dtype))
    n_params = len(in_names)
    n_outs = len(out_avals)
    all_names = in_names + out_names
    if partition_name is not None:
        all_names = all_names + [partition_name]
    donate = tuple(range(n_params, n_params + n_outs))
    n_cores = len(devices)

    def _body(*args):
        operands = list(args)
        if partition_name is not None:
            operands.append(partition_id_tensor())
        outs = _bass_exec_p.bind(
            *operands,
            out_avals=tuple(out_avals),
            in_names=tuple(all_names),
            out_names=tuple(out_names),
            lowering_input_output_aliases=(),
            sim_require_finite=True,
            sim_require_nnan=True,
            nc=nc,
        )
        return tuple(outs)

    mesh = Mesh(np.asarray(devices), ("core",))
    in_specs = (PartitionSpec("core"),) * (n_params + n_outs)
    out_specs = (PartitionSpec("core"),) * n_outs
    sharded = jax.jit(
        shard_map(_body, mesh=mesh, in_specs=in_specs, out_specs=out_specs,
                  check_rep=False),
        donate_argnums=donate, keep_unused=True)

    def run(in_maps):
        per_core = [[np.asarray(m[name]) for name in in_names] for m in in_maps]
        concat_in = [
            np.concatenate([per_core[c][i] for c in range(n_cores)], axis=0)
            for i in range(n_params)
        ]
        concat_zeros = [
            np.zeros((n_cores * z.shape[0], *z.shape[1:]), z.dtype)
            for z in zero_outs
        ]
        return sharded(*concat_in, *concat_zeros)  # async jax arrays

    def gather(out_arrs):
        return [
            {name: np.asarray(out_arrs[i]).reshape(n_cores, *out_avals[i].shape)[c]
             for i, name in enumerate(out_names)}
            for c in range(n_cores)
        ]

    return run, gather, out_names


def _tiles_for(group_starts):
    tiles = []
    for a in group_starts:
        tiles.extend(range(a, a + 4))
    return tiles


def _get_runners():
    if "runA" not in _CACHE:
        import jax
        devs = jax.devices()
        ncA = _build(GROUPS_A, KV_CHUNKS_A, Q_CHUNKS_A)
        ncB = _build(GROUPS_B, KV_CHUNKS_B, Q_CHUNKS_B)
        _CACHE["runA"] = _make_runner(ncA, devs[0:4])
        _CACHE["runB"] = _make_runner(ncB, devs[4:8])
    return _CACHE["runA"], _CACHE["runB"]


def kernel(x, Wqkv, Wproj, _trace_ctx=None):
    x = np.ascontiguousarray(x, dtype=np.float32)
    xT = np.ascontiguousarray(np.transpose(x, (0, 2, 1)))  # [B, C, T]
    xT_bf = xT.astype(ml_dtypes.bfloat16)
    xT_f8 = xT.astype(ml_dtypes.float8_e4m3)
    wqkv_f8 = np.asarray(Wqkv, dtype=np.float32).astype(ml_dtypes.float8_e4m3)
    wqkv_bf = np.asarray(Wqkv, dtype=np.float32).astype(ml_dtypes.bfloat16)
    wproj_bf = np.asarray(Wproj, dtype=np.float32).astype(ml_dtypes.bfloat16)
    wproj_f8 = np.asarray(Wproj, dtype=np.float32).astype(ml_dtypes.float8_e4m3)

    (runA, gatherA, _), (runB, gatherB, _) = _get_runners()

    maps = [{"xT_in": xT_bf[b], "x8_in": xT_f8[b], "wqkv": wqkv_bf,
             "w8": wqkv_f8, "wproj": wproj_bf, "wp8": wproj_f8}
            for b in range(B)]

    import contextlib
    ctx = _trace_ctx if _trace_ctx is not None else contextlib.nullcontext()
    with ctx:
        outA = runA(maps)
        outB = runB(maps)
        resA = gatherA(outA)
        resB = gatherB(outB)

    tilesA = _tiles_for(GROUPS_A)
    tilesB = _tiles_for(GROUPS_B)
    out = np.empty((B, T, C), dtype=np.float32)
    for b in range(B):
        for slot, tile_i in enumerate(tilesA):
            out[b, 128 * tile_i:128 * (tile_i + 1)] = \
                resA[b]["y"][128 * slot:128 * (slot + 1)]
        for slot, tile_i in enumerate(tilesB):
            out[b, 128 * tile_i:128 * (tile_i + 1)] = \
                resB[b]["y"][128 * slot:128 * (slot + 1)]
    return out
